# revision 4
# baseline (speedup 1.0000x reference)
"""Trainium2 Bass kernel for GCN+RNN (nn_GCNN_RNN_32461362823865).

Strategy (v4.1):
  - Host: dense normalized adjacency A^T (fp16, 3072-padded), fold
    W2 = W @ W_ih.T and c0 = b @ W_ih.T + b_ih + b_hh, pre-transpose x.
  - One sync DMA queue ordering: x0-5, then A blocks with x6-15
    interleaved (2 A : 1 x). U-block-0 (dc0-5, 6 psum banks) chases A's
    arrival kb-by-kb; all of z (x @ W2) runs in the same window (zp pool,
    2 banks, batches of 2 samples x 4 kb; psum->fp16 copies alternate
    DVE / ScalarE-Copy). z batches and chase MMs are emitted merged by
    estimated data-arrival time so the PE queue never blocks long and
    the pstate stays hot.
  - 7 a2a rounds, round r triggered after M-block r; RNN steps weave
    into blocks 2-6 (24 slots/block, after odd kb in both passes);
    tail = steps 112-127 (split column halves over two psum banks).
  - RNN ring [128 part, 8 slots x 384]: rows 0:50 = h (ScalarE tanh),
    rows 64:114 = U (DMA'd from a2a output on the gpsimd queue, gated
    on round triggers). One 128-contraction MM per step.

  Sample->core map: core c, round r holds global samples
  BOFF[r] + SR[r]*c + s4, SR = (2,3,2,3,2,3,1).
"""
import numpy as np

import concourse.bacc as bacc
import concourse.mybir as mybir
from concourse import tile
from concourse.bass_utils import run_bass_kernel_spmd

# ---- problem constants (hardcoded per contract) ----
N = 3070          # nodes
NP = 3072         # padded nodes (24 * 128, 8 * 384)
F = 128           # input features
J = 50            # folded feature dim (= RNN hidden)
B = 128           # batch (RNN sequence length)
NCORES = 8
S = B // NCORES   # samples per core = 16
NPC = NP // NCORES  # nodes per core = 384
KB = NP // 128    # 24 contraction blocks
SJ = S * J        # 800 U^T rows per core
NMB = 7           # M-blocks of 128 rows (last = 32)

SR = (2, 3, 2, 3, 2, 3, 1)       # samples per round per core
R = len(SR)
BOFF = [0, 16, 40, 56, 80, 96, 120]   # global step offset per round
ROFF = [0, 100, 250, 350, 500, 600, 750, 800]  # U^T row offset per round
RJ = [SR[r] * J for r in range(R)]
TRIG_BLK = {r: r for r in range(R)}  # round r triggers after block r
DCP = [list(range(6)), [6, 7]]   # dest-core passes (6+2 psum banks)

RING = 8          # rnn ring slots
TAIL_B = 112      # steps >= TAIL_B use the split (2 MM + 2 ACT) form
PRE = 7           # U prestage distance (steps ahead)

# weave schedule: M-block -> rnn steps woven into it
WEAVE = {2: list(range(0, 16)), 3: list(range(16, 40)),
         4: list(range(40, 64)), 5: list(range(64, 88)),
         6: list(range(88, 112))}

F16 = mybir.dt.float16
F32 = mybir.dt.float32
TANH = mybir.ActivationFunctionType.Tanh
COPY = mybir.ActivationFunctionType.Copy

_PROGRAM_CACHE = {}


def _step_rc(b):
    """global step -> (round, src core, sample-within-block)."""
    r = 0
    while r + 1 < R and b >= BOFF[r + 1]:
        r += 1
    q = b - BOFF[r]
    return r, q // SR[r], q % SR[r]


# writeback groups aligned so ring slots never wrap mid-group:
# out[b] lives in slot (b+1) % RING rows 0:50.
WB_GROUPS = [(0, 3)] + [(3 + 4 * i, 4) for i in range(31)] + [(127, 1)]
WB_AFTER = {b0 + g - 1: (b0, g) for b0, g in WB_GROUPS}

# DMA-order bookkeeping: x0-5 first, then A blocks with x6-15 after
# every 2nd A block. Estimated arrival times (us) pace the merged
# emission of z batches and chase MMs.
_TX = 2.24   # one x sample transfer
_TA = 2.29   # one A block transfer


def _arrivals():
    t = 0.0
    tx, ta = {}, {}
    for s in range(6):
        t += _TX
        tx[s] = t
    nx = 6
    kb = 0
    while kb < KB:
        for _ in range(2):
            if kb < KB:
                t += _TA
                ta[kb] = t
                kb += 1
        if nx < S:
            t += _TX
            tx[nx] = t
            nx += 1
    return tx, ta


def _build_program():
    if "nc" in _PROGRAM_CACHE:
        return _PROGRAM_CACHE["nc"]
    nc = bacc.Bacc("TRN2", target_bir_lowering=False, debug=False,
                   num_devices=NCORES)

    xT = nc.dram_tensor("xT", [S, F, N], F16, kind="ExternalInput")
    at = nc.dram_tensor("at", [NP, NP], F16, kind="ExternalInput")
    w2 = nc.dram_tensor("w2", [F, J], F16, kind="ExternalInput")
    ws = nc.dram_tensor("ws", [128, J], F16, kind="ExternalInput")
    c0 = nc.dram_tensor("c0", [J, 1], F32, kind="ExternalInput")
    h0T = nc.dram_tensor("h0T", [J, NPC], F16, kind="ExternalInput")
    out = nc.dram_tensor("out", [B, J, NPC], F16, kind="ExternalOutput")

    tx_est, ta_est = _arrivals()

    with tile.TileContext(nc) as tc:
        with (
            tc.tile_pool(name="consts", bufs=1) as consts,
            tc.tile_pool(name="persist", bufs=1) as persist,
            tc.tile_pool(name="xin", bufs=3) as xin,
            tc.tile_pool(name="stg", bufs=1) as stg_pool,
            tc.tile_pool(name="dram", bufs=1, space="DRAM") as dram,
        ):
            w2_sb = consts.tile([F, J], F16, tag="w2_sb")
            ws_sb = consts.tile([128, J], F16, tag="ws_sb")
            c0_sb = consts.tile([J, 1], F32, tag="c0_sb")
            nc.scalar.dma_start(w2_sb[:], w2[:])
            nc.scalar.dma_start(ws_sb[:], ws[:])
            nc.scalar.dma_start(c0_sb[:], c0[:])

            at_sb = persist.tile([128, KB * NP], F16, tag="at_sb")
            z_sb = persist.tile([128, KB * SJ], F16, tag="z_sb")
            ring = persist.tile([128, RING * NPC], F16, tag="ring")
            nc.vector.memset(ring[:], 0.0)
            nc.scalar.dma_start(ring[0:J, 0:NPC], h0T[:])  # h0 -> slot 0

            a2a_in = [dram.tile([NCORES * RJ[r], NPC], F16, name=f"a2ai_{r}")
                      for r in range(R)]
            a2a_out = [dram.tile([NCORES * RJ[r], NPC], F16, name=f"a2ao_{r}")
                       for r in range(R)]
            warm_in = dram.tile([NCORES, 8192], F16, name="warm_in")
            warm_out = dram.tile([NCORES, 8192], F16, name="warm_out")

            # dummy collective: absorbs barrier + mesh warmup during the
            # DMA-bound chase window
            nc.gpsimd.collective_compute(
                "AllToAll", mybir.AluOpType.bypass,
                replica_groups=[list(range(NCORES))],
                ins=[warm_in.opt()], outs=[warm_out.opt()])

            state = {}
            trig_done = [False] * R
            pending_pre = []
            xbigs = {}

            def load_x(s):
                xb = xin.tile([F, NP], F16, tag="xbig", name=f"xbig_{s}")
                xbigs[s] = xb
                nc.sync.dma_start(xb[:, 0:N], xT[s])
                nc.vector.memset(xb[:, N:NP], 0.0)

            def _do_prestage(b):
                r, c, s4 = _step_rc(b)
                slot = b % RING
                row = c * RJ[r] + s4 * J
                nc.gpsimd.dma_start(
                    ring[64:64 + J, slot * NPC:(slot + 1) * NPC],
                    a2a_out[r][row:row + J, :])

            def prestage_u(b):
                if trig_done[_step_rc(b)[0]]:
                    _do_prestage(b)
                else:
                    pending_pre.append(b)

            def trig(r):
                nc.gpsimd.collective_compute(
                    "AllToAll", mybir.AluOpType.bypass,
                    replica_groups=[list(range(NCORES))],
                    ins=[a2a_in[r].opt()],
                    outs=[a2a_out[r].opt()])
                trig_done[r] = True
                ready = [b for b in pending_pre if trig_done[_step_rc(b)[0]]]
                for b in ready:
                    pending_pre.remove(b)
                    _do_prestage(b)

            def rnn_step(b):
                slot = b % RING
                nslot = (b + 1) % RING
                if b + PRE < B:
                    prestage_u(b + PRE)
                rhs = ring[:, slot * NPC:(slot + 1) * NPC]
                dst = ring[0:J, nslot * NPC:(nslot + 1) * NPC]
                if b < TAIL_B:
                    pp = state["pp"].tile([J, NPC], F32, tag="pp",
                                          name=f"pp_{b}")
                    nc.tensor.matmul(pp[:], ws_sb[:], rhs,
                                     start=True, stop=True)
                    nc.scalar.activation(dst, pp[:], TANH,
                                         bias=c0_sb[:, 0:1])
                else:
                    H = NPC // 2
                    for half in range(2):
                        pph = state["pp"].tile([J, NPC], F32, tag="pp",
                                               name=f"pp_{b}_{half}")
                        nc.tensor.matmul(pph[:, 0:H], ws_sb[:],
                                         rhs[:, half * H:(half + 1) * H],
                                         start=True, stop=True)
                        nc.scalar.activation(
                            dst[:, half * H:(half + 1) * H], pph[:, 0:H],
                            TANH, bias=c0_sb[:, 0:1])
                if b in WB_AFTER:
                    b0, g = WB_AFTER[b]
                    s0 = (b0 + 1) % RING
                    nc.sync.dma_start(
                        out[b0:b0 + g].rearrange("g j n -> j g n"),
                        ring[0:J, s0 * NPC:(s0 + g) * NPC].rearrange(
                            "j (g n) -> j g n", g=g))

            def stage(k, dcs, st):
                """a2a staging DMAs for M-block k, dest cores `dcs`
                (consecutive), source st [mrows, len(dcs)*NPC] fp16."""
                row0 = k * 128
                mrows = min(128, SJ - row0)
                nd = len(dcs)
                dc0 = dcs[0]
                for r in range(R):
                    lo = max(row0, ROFF[r])
                    hi = min(row0 + mrows, ROFF[r + 1])
                    if lo >= hi:
                        continue
                    nc.sync.dma_start(
                        a2a_in[r].rearrange("(dc rw) n -> rw dc n",
                                            dc=NCORES)[
                            lo - ROFF[r]:hi - ROFF[r], dc0:dc0 + nd, :],
                        st[lo - row0:hi - row0, :].rearrange(
                            "rw (dc n) -> rw dc n", dc=nd))

            def stage_psums(k, dcs, psums):
                """psums (f32) -> fp16 staging tiles -> a2a_in, chunks of
                <=2 consecutive dest cores."""
                row0 = k * 128
                mrows = min(128, SJ - row0)
                i = 0
                while i < len(dcs):
                    chunk = dcs[i:i + 2]
                    st = stg_pool.tile([128, 2 * NPC], F16, tag="st",
                                       name=f"st_{k}_{chunk[0]}")
                    for di, dc in enumerate(chunk):
                        nc.vector.tensor_copy(
                            st[0:mrows, di * NPC:(di + 1) * NPC],
                            psums[dc][0:mrows, :])
                    stage(k, chunk, st[0:mrows, 0:len(chunk) * NPC])
                    i += 2

            # ---- psum pools for the startup window (LIFO stack) ----
            zp_ctx = tc.tile_pool(name="zpsum", bufs=2, space="PSUM")
            zp = zp_ctx.__enter__()
            chase_ctx = tc.tile_pool(name="chase", bufs=6, space="PSUM")
            chase = chase_ctx.__enter__()

            copy_alt = [0]

            def zbatch(p, i):
                """z for samples (2p, 2p+1), contraction blocks 4i..4i+3."""
                zt = zp.tile([128, 400], F32, tag="zp", name=f"zp_{p}_{i}")
                for si in range(2):
                    for jj in range(4):
                        kb = 4 * i + jj
                        nc.tensor.matmul(
                            zt[:, jj * 100 + si * J:jj * 100 + (si + 1) * J],
                            xbigs[2 * p + si][:, kb * 128:(kb + 1) * 128],
                            w2_sb[:], start=True, stop=True)
                for jj in range(4):
                    kb = 4 * i + jj
                    dst = z_sb[:, kb * SJ + 2 * p * J:kb * SJ + (2 * p + 2) * J]
                    src = zt[:, jj * 100:(jj + 1) * 100]
                    if copy_alt[0] % 2 == 0:
                        nc.vector.tensor_copy(dst, src)
                    else:
                        nc.scalar.activation(dst, src, COPY)
                    copy_alt[0] += 1

            # ================= fused startup: chase A's DMA ==============
            with nc.named_scope("startup"):
                # DMA emission order IS queue order on sync: x0-5, then
                # A blocks with one x after every 2nd A.
                for s in range(6):
                    load_x(s)
                nx = 6
                kb = 0
                while kb < KB:
                    for _ in range(2):
                        if kb < KB:
                            nc.sync.dma_start(
                                at_sb[:, kb * NP:(kb + 1) * NP],
                                at[kb * 128:(kb + 1) * 128, :])
                            kb += 1
                    if nx < S:
                        load_x(nx)
                        nx += 1

                cps = {dc: chase.tile([128, NPC], F32, tag="ch",
                                      name=f"ch_{dc}")
                       for dc in range(6)}

                # merged emission of z batches and chase MMs by
                # estimated arrival time
                events = []
                for p in range(8):
                    tz = tx_est[2 * p + 1]
                    for i in range(6):
                        events.append((tz + 0.9 * i, 0, ("z", p, i)))
                for kb in range(KB):
                    events.append((ta_est[kb], 1, ("ch", kb)))
                events.sort()
                for _, _, ev in events:
                    if ev[0] == "z":
                        zbatch(ev[1], ev[2])
                    else:
                        kb = ev[1]
                        for dc in range(6):
                            nc.tensor.matmul(
                                cps[dc][:],
                                z_sb[:, kb * SJ:kb * SJ + 128],
                                at_sb[:, kb * NP + dc * NPC:
                                      kb * NP + (dc + 1) * NPC],
                                start=(kb == 0), stop=(kb == KB - 1))

                stage_psums(0, list(range(6)), cps)

            chase_ctx.__exit__(None, None, None)
            zp_ctx.__exit__(None, None, None)
            upsum_ctx = tc.tile_pool(name="upsum", bufs=6, space="PSUM")
            upsum = upsum_ctx.__enter__()
            pp_ctx = tc.tile_pool(name="p3psum", bufs=2, space="PSUM")
            state["pp"] = pp_ctx.__enter__()

            with nc.named_scope("ummphase"):
                # finish block 0: dc6,7 (full 24-kb passes, A resident)
                ps67 = {dc: upsum.tile([128, NPC], F32, tag="up",
                                       name=f"up_0_{dc}")
                        for dc in (6, 7)}
                for kb in range(KB):
                    for dc in (6, 7):
                        nc.tensor.matmul(
                            ps67[dc][:], z_sb[:, kb * SJ:kb * SJ + 128],
                            at_sb[:, kb * NP + dc * NPC:
                                  kb * NP + (dc + 1) * NPC],
                            start=(kb == 0), stop=(kb == KB - 1))
                stage_psums(0, [6, 7], ps67)
                trig(0)

                for b in range(PRE):
                    prestage_u(b)

                # ================= U-phase blocks 1-6 ===================
                for k in range(1, NMB):
                    row0 = k * 128
                    mrows = min(128, SJ - row0)
                    steps = list(WEAVE.get(k, []))
                    w1, wrest = steps[:12], steps[12:]
                    for pi, dcs in enumerate(DCP):
                        psums = {dc: upsum.tile([128, NPC], F32, tag="up",
                                                name=f"up_{k}_{dc}")
                                 for dc in dcs}
                        wq = w1 if pi == 0 else wrest
                        for kb in range(KB):
                            for dc in dcs:
                                nc.tensor.matmul(
                                    psums[dc][0:mrows, :],
                                    z_sb[:, kb * SJ + row0:
                                         kb * SJ + row0 + mrows],
                                    at_sb[:, kb * NP + dc * NPC:
                                          kb * NP + (dc + 1) * NPC],
                                    start=(kb == 0), stop=(kb == KB - 1))
                            if kb % 2 == 1 and wq:
                                rnn_step(wq.pop(0))
                        stage_psums(k, dcs, psums)
                        while wq:
                            rnn_step(wq.pop(0))
                    for r, blk in TRIG_BLK.items():
                        if blk == k:
                            trig(r)

            # ================= rnn tail ==================================
            with nc.named_scope("rnn"):
                for b in range(TAIL_B, B):
                    rnn_step(b)

            pp_ctx.__exit__(None, None, None)
            upsum_ctx.__exit__(None, None, None)

    nc.compile()
    _PROGRAM_CACHE["nc"] = nc
    return nc


def _host_prep(x_in, edge_index, edge_weight, W, b, W_ih, W_hh, b_ih, b_hh, h0):
    """Build per-core input maps (all numpy, no device work)."""
    edge_index = np.asarray(edge_index).astype(np.int64)
    # exact reference remap: rank among unique ids (size=N, fill=2**30)
    uniq = np.unique(edge_index)
    if uniq.size < N:
        uniq = np.concatenate([uniq, np.full(N - uniq.size, 2 ** 30, np.int64)])
    else:
        uniq = uniq[:N]
    ei = np.searchsorted(uniq, edge_index)
    src, dst = ei[0], ei[1]

    ew = np.asarray(edge_weight, np.float64)
    deg = np.zeros(N, np.float64)
    np.add.at(deg, dst, ew)
    deg += 1.0  # self loops (weight 1)
    dinv = np.where(deg > 0, 1.0 / np.sqrt(deg), 0.0)

    AT = np.zeros((NP, NP), np.float32)
    np.add.at(AT, (src, dst), (dinv[src] * ew * dinv[dst]).astype(np.float32))
    idx = np.arange(N)
    AT[idx, idx] += (dinv * dinv).astype(np.float32)
    AT16 = AT.astype(np.float16)

    W = np.asarray(W, np.float32)
    W_ih = np.asarray(W_ih, np.float32)
    W2 = (W.astype(np.float64) @ W_ih.T.astype(np.float64)).astype(np.float16)
    c0 = (np.asarray(b, np.float32) @ W_ih.T + np.asarray(b_ih, np.float32)
          + np.asarray(b_hh, np.float32)).astype(np.float32).reshape(J, 1)
    ws = np.zeros((128, J), np.float32)
    ws[0:J] = np.asarray(W_hh, np.float32).T
    ws[64:64 + J] = np.eye(J, dtype=np.float32)
    ws = ws.astype(np.float16)

    x_in = np.asarray(x_in, np.float32)
    h0 = np.asarray(h0, np.float32)
    h0p = np.zeros((NP, J), np.float16)
    h0p[:N] = h0.astype(np.float16)

    in_maps = []
    for c in range(NCORES):
        samples = [BOFF[r] + SR[r] * c + s4
                   for r in range(R) for s4 in range(SR[r])]
        xc = x_in[samples]                                # (S, N, F)
        xTc = np.ascontiguousarray(
            xc.transpose(0, 2, 1)).astype(np.float16)     # (S, F, N)
        h0Tc = np.ascontiguousarray(
            h0p[c * NPC:(c + 1) * NPC].T)                 # (J, NPC)
        in_maps.append({"xT": xTc, "at": AT16, "w2": W2, "ws": ws,
                        "c0": c0, "h0T": h0Tc})
    return in_maps


def _assemble(results):
    parts = []
    for c in range(NCORES):
        o = results[c]["out"]                 # (B, J, NPC) fp16
        parts.append(np.ascontiguousarray(o.transpose(0, 2, 1)))  # (B, NPC, J)
    full = np.concatenate(parts, axis=1)      # (B, NP, J)
    return full[:, :N, :].astype(np.float32)


def run_internal(inputs, trace=False, trace_cores=None):
    nc = _build_program()
    in_maps = _host_prep(**inputs)
    res = run_bass_kernel_spmd(nc, in_maps, list(range(NCORES)), trace=trace,
                               trace_cores=trace_cores)
    return _assemble(res.results), res


def kernel(**inputs) -> np.ndarray:
    out, _ = run_internal(inputs, trace=False)
    return out
